# revision 10
# baseline (speedup 1.0000x reference)
"""Trainium2 Bass kernel for nn_Net_3152505995417 (gnn_message_passing).

Closed-form reformulation: with T the incidence matrix of a simple graph,
  node conv:  (T diag(d) T^T) * adj_v  ==  A with A[i,j] = d[edge(i,j)], 0 diag
  edge conv:  M = (T^T diag(dv) T) * adj_e has M[e,f] = dv[shared node],
              col-max(f=(k,l)) = max(dv[k], dv[l], 0)   (complete graph),
              row e=(i,j) of (M/colmax) @ G = dv_i*(S_i - Gn_e) + dv_j*(S_j - Gn_e)
              with Gn = G / (colmax + eps), S = T @ Gn.
So the E x E matrix is never materialized: everything lives in a dense
[N, N] node-pair layout (slots (i,j) and (j,i) both carry edge {i,j};
diagonal slots are zero). Gathers/scatters become row/column broadcasts
(PE ones-matmuls) and free-dim row-sums of [116, 116] tiles.

All inputs arrive in one packed [128, 1016] slab (two DMAs); the full
model runs replicated on each of the 8 NeuronCores (total work is a few
hundred KB — replication beats collective latency); core 0's output is
returned.
"""

import numpy as np

N = 116
E = N * (N - 1) // 2
HID = 64
EDIM = 5
OUT = 4
ENC = HID + N // 2
EPS = 1e-10

# packed slab column offsets
C_EA = 0                 # [0:116, 0:580]   ea dense, k-major (f = k*N + j)
C_SVEC = 1016            # [0, 1016:1056]   p1|p2|be|We.flat
C_ENCT = 580             # [0:122, 580:696]
C_WENC = 696             # [0:122, 696:760]
C_W1 = 760               # [0:64]
C_W2 = 824
C_WL = 888               # [0:64, 888:892]
C_MASK = 892             # [0:116, 892:1008]
C_BENC = 1008
C_B1 = 1009
C_B2 = 1010
C_PET = 1011
C_BL = 1012              # [0:4]
SLAB_W = 1056
SPLIT = 580              # DMA A = cols [0:580], DMA B = cols [580:1016]

# plane-boundary chunking of the 5*116 edge-conv slab (PE N<=512, PSUM bank)
CH = [(0, 232), (232, 580)]

_CACHE = {}


def _split_excess_waits(nc, mybir, max_waits=1):
    """Workaround: this walrus build accepts only one sync-wait per
    instruction (setupSyncWait: "Too many sync wait commands"). Move excess
    waits onto chained NoOps on the same engine immediately before the
    instruction; sequencer semantics are unchanged."""
    for fn in nc.m.functions:
        for blk in fn.blocks:
            insts = blk.instructions
            new, changed = [], False
            for ins in insts:
                si = ins.sync_info
                waits = list(si.on_wait) if si is not None else []
                if len(waits) > max_waits:
                    while len(waits) > max_waits:
                        chunk, waits = waits[:1], waits[1:]
                        nop = mybir.InstNoOp(
                            name=nc.get_next_instruction_name(),
                            engine=ins.engine,
                            sync_info=mybir.SyncInfo(on_wait=chunk, on_update=[]),
                            bass_nofuse=True,
                        )
                        new.append(nop)
                    si.on_wait = waits
                    changed = True
                new.append(ins)
            if changed:
                blk.instructions = new


def _build():
    import concourse.bass as bass
    import concourse.tile as tile
    from concourse import mybir

    f32 = mybir.dt.float32
    A = mybir.AluOpType
    Relu = mybir.ActivationFunctionType.Relu

    nc = bass.Bass("TRN2", target_bir_lowering=False, num_devices=8)

    slabA_d = nc.declare_dram_parameter("slabA", [128, SPLIT], f32, isOutput=False)
    slabB_d = nc.declare_dram_parameter(
        "slabB", [128, SLAB_W - SPLIT], f32, isOutput=False
    )
    out_d = nc.declare_dram_parameter("out", [OUT, 1], f32, isOutput=True)

    with tile.TileContext(nc) as tc:
        with (
            tc.tile_pool(name="sb", bufs=1) as sb,
            tc.tile_pool(name="pm", bufs=2) as pm,
            tc.tile_pool(name="ps", bufs=3, space="PSUM") as ps,
            tc.tile_pool(name="ps2", bufs=2, space="PSUM") as ps2,
        ):
            slab = sb.tile([128, SLAB_W], f32, tag="slab")
            nc.sync.dma_start(out=slab[:, 0:SPLIT], in_=slabA_d[:])
            nc.sync.dma_start(out=slab[:, SPLIT:SLAB_W], in_=slabB_d[:])

            ea = slab[0:N, 0:EDIM * N]
            svec = slab[0:1, C_SVEC:C_SVEC + 40]
            encT = slab[0:ENC, C_ENCT:C_ENCT + N]
            Wenc = slab[0:ENC, C_WENC:C_WENC + HID]
            W1 = slab[0:HID, C_W1:C_W1 + HID]
            W2 = slab[0:HID, C_W2:C_W2 + HID]
            Wl = slab[0:HID, C_WL:C_WL + OUT]
            mask = slab[0:N, C_MASK:C_MASK + N]
            benc = slab[0:HID, C_BENC:C_BENC + 1]
            b1 = slab[0:HID, C_B1:C_B1 + 1]
            b2 = slab[0:HID, C_B2:C_B2 + 1]
            peT = slab[0:HID, C_PET:C_PET + 1]
            bl = slab[0:OUT, C_BL:C_BL + 1]

            ones_row = sb.tile([1, N], f32, tag="ones_row")
            nc.vector.memset(ones_row[:], 1.0)
            ones_col = sb.tile([N, 1], f32, tag="ones_col")
            nc.vector.memset(ones_col[:], 1.0)
            # pre-warm the ACT Relu table while DMAs are in flight
            warm = sb.tile([1, 1], f32, tag="warm")
            nc.scalar.activation(warm[:], ones_row[:, 0:1], Relu)

            # ---- broadcast the small row-vector params to all partitions ----
            svecB_ps = ps.tile([N, 40], f32, tag="ps")
            nc.tensor.matmul(svecB_ps[:], ones_row[:], svec, start=True, stop=True)
            svecB = sb.tile([N, 40], f32, tag="svecB")
            nc.vector.tensor_copy(svecB[:], svecB_ps[:])
            p1B = svecB[:, 0:5]
            p2B = svecB[:, 5:10]
            beB = svecB[:, 10:15]
            # We[k, m] at column 15 + k*5 + m

            # ---- x = enc @ W_enc + b_enc  (kept transposed: [HID, N]) ----
            xT_ps = ps.tile([HID, N], f32, tag="ps")
            nc.tensor.matmul(xT_ps[:], Wenc, encT, start=True, stop=True)
            xT = sb.tile([HID, N], f32, tag="xT")
            nc.vector.tensor_scalar_add(xT[:], xT_ps[:], benc)

            # ---- A1 = d1 (dense pair layout; diag slots already zero) ----
            d1tmp = sb.tile([N, EDIM * N], f32, tag="d1tmp")
            nc.vector.tensor_tensor(
                d1tmp[:].rearrange("p (k j) -> p k j", k=EDIM),
                ea.rearrange("p (k j) -> p k j", k=EDIM),
                p1B[:, :, None].to_broadcast([N, EDIM, N]),
                A.mult,
            )
            d1 = sb.tile([N, N], f32, tag="d1")
            nc.vector.tensor_reduce(
                d1[:], d1tmp[:].rearrange("p (k j) -> p j k", k=EDIM),
                mybir.AxisListType.X, A.add,
            )

            # ---- node conv 1: x1T = relu((A1 @ (x @ W1) + b1)^T) ----
            xW1_ps = ps.tile([N, HID], f32, tag="ps")
            nc.tensor.matmul(xW1_ps[:], xT[:], W1, start=True, stop=True)
            xW1 = sb.tile([N, HID], f32, tag="xW1")
            nc.vector.tensor_copy(xW1[:], xW1_ps[:])
            x1T_ps = ps.tile([HID, N], f32, tag="ps")
            nc.tensor.matmul(x1T_ps[:], xW1[:], d1[:], start=True, stop=True)
            x1T = sb.tile([HID, N], f32, tag="x1T")
            nc.scalar.activation(x1T[:], x1T_ps[:], Relu, bias=b1)

            # ---- dv = x1 @ pe^T, as row [1,N] and column [N,1] ----
            dvr_ps = ps.tile([1, N], f32, tag="ps")
            nc.tensor.matmul(dvr_ps[:], peT, x1T[:], start=True, stop=True)
            dv_row = sb.tile([1, N], f32, tag="dv_row")
            nc.vector.tensor_copy(dv_row[:], dvr_ps[:])
            dvT_ps = ps.tile([N, 1], f32, tag="ps")
            nc.tensor.matmul(dvT_ps[:], x1T[:], peT, start=True, stop=True)
            dvT = sb.tile([N, 1], f32, tag="dvT")
            nc.vector.tensor_copy(dvT[:], dvT_ps[:])
            dvROW_ps = ps.tile([N, N], f32, tag="ps")
            nc.tensor.matmul(dvROW_ps[:], ones_row[:], dv_row[:], start=True, stop=True)

            # negsumdv[i,j] = -(dv_i + dv_j);  cmeps = max(dv_i,dv_j,0)+eps
            negsumdv = sb.tile([N, N], f32, tag="negsumdv")
            nc.vector.tensor_scalar(
                negsumdv[:], dvROW_ps[:], dvT[:, 0:1], -1.0, A.add, A.mult
            )
            cmeps = sb.tile([N, N], f32, tag="cmeps")
            nc.vector.tensor_scalar(
                cmeps[:], dvROW_ps[:], dvT[:, 0:1], 0.0, A.max, A.max
            )
            nc.vector.tensor_scalar_add(cmeps[:], cmeps[:], EPS)
            nc.vector.reciprocal(cmeps[:], cmeps[:])

            # ---- edge conv (plane-major slabs, f = m*N + j) ----
            eR = sb.tile([N, EDIM * N], f32, tag="eR")
            nc.scalar.activation(eR[:], ea, Relu)

            # G[:, (m,j)] = sum_k eR_k[:, j] * We[k, m]; eR_k broadcast along
            # m (outer step-0), We row broadcast along j (inner step-0).
            def eRk_b(k):
                return eR[:, k * N:(k + 1) * N][:, None, :].to_broadcast(
                    [N, EDIM, N]
                )

            def WeB_b(k):
                return svecB[:, 15 + k * 5:15 + k * 5 + 5][:, :, None].to_broadcast(
                    [N, EDIM, N]
                )

            G = sb.tile([N, EDIM * N], f32, tag="G")
            G3 = G[:].rearrange("p (m j) -> p m j", m=EDIM)
            Gt = sb.tile([N, EDIM * N], f32, tag="Gt")
            Gt3 = Gt[:].rearrange("p (m j) -> p m j", m=EDIM)
            Gg = sb.tile([N, EDIM * N], f32, tag="Gg")
            Gg3 = Gg[:].rearrange("p (m j) -> p m j", m=EDIM)
            nc.vector.tensor_tensor(G3, eRk_b(0), WeB_b(0), A.mult)
            for k in (1, 2, 3):
                nc.vector.tensor_tensor(Gt3, eRk_b(k), WeB_b(k), A.mult)
                nc.vector.tensor_tensor(G3, G3, Gt3, A.add)
            nc.gpsimd.tensor_tensor(Gg3, eRk_b(4), WeB_b(4), A.mult)
            nc.vector.tensor_tensor(G3, G3, Gg3, A.add)

            # Gn = G * (1/cmeps) and S[i,m] = sum_j Gn_m[i,j], per chunk so
            # the S_row/U/z chain for chunk 0 overlaps chunk 1's compute
            Gn = sb.tile([N, EDIM * N], f32, tag="Gn")
            S_all = sb.tile([N, EDIM], f32, tag="S_all")
            for c0, c1 in CH:
                nm = (c1 - c0) // N
                m0 = c0 // N
                nc.vector.tensor_tensor(
                    Gn[:, c0:c1].rearrange("p (m j) -> p m j", m=nm),
                    G[:, c0:c1].rearrange("p (m j) -> p m j", m=nm),
                    cmeps[:, None, :].to_broadcast([N, nm, N]),
                    A.mult,
                )
                nc.vector.tensor_reduce(
                    S_all[:, m0:m0 + nm],
                    Gn[:, c0:c1].rearrange("p (m j) -> p m j", m=nm),
                    mybir.AxisListType.X, A.add,
                )
            # t1b[:, m] = dv_i * S[i, m] + be_m
            t1b = sb.tile([N, EDIM], f32, tag="t1b")
            nc.vector.scalar_tensor_tensor(
                t1b[:], S_all[:], dvT[:, 0:1], beB, A.mult, A.add
            )

            # q = Gn * negsumdv (broadcast along m) -- on GpSimd (idle engine)
            q = sb.tile([N, EDIM * N], f32, tag="q")
            nc.gpsimd.tensor_tensor(
                q[:].rearrange("p (m j) -> p m j", m=EDIM),
                Gn[:].rearrange("p (m j) -> p m j", m=EDIM),
                negsumdv[:, None, :].to_broadcast([N, EDIM, N]),
                A.mult,
            )

            # S as rows (colsum of symmetric Gn), then U[i, (m,j)] = dv_j*S[j,m]
            z = sb.tile([N, EDIM * N], f32, tag="z")
            for c0, c1 in CH:
                w = c1 - c0
                nm = w // N
                Srow_ps = ps2.tile([1, w], f32, tag="psrow")
                nc.tensor.matmul(
                    Srow_ps[:], ones_col[:], Gn[:, c0:c1], start=True, stop=True
                )
                u = pm.tile([1, w], f32, tag="u")
                nc.vector.tensor_tensor(
                    u[:].rearrange("p (m j) -> p m j", m=nm),
                    dv_row[:, None, :].to_broadcast([1, nm, N]),
                    Srow_ps[:].rearrange("p (m j) -> p m j", m=nm),
                    A.mult,
                )
                U_ps = ps2.tile([N, w], f32, tag="psU")
                nc.tensor.matmul(U_ps[:], ones_row[:], u[:], start=True, stop=True)
                nc.vector.tensor_tensor(z[:, c0:c1], q[:, c0:c1], U_ps[:], A.add)

            # e2_m = relu(z_m + t1b_m)
            e2 = sb.tile([N, EDIM * N], f32, tag="e2")
            for m in range(EDIM):
                nc.scalar.activation(
                    e2[:, m * N:(m + 1) * N], z[:, m * N:(m + 1) * N], Relu,
                    bias=t1b[:, m:m + 1],
                )

            # ---- A2 = (e2 @ p2^T) * mask ----
            d2tmp = sb.tile([N, EDIM * N], f32, tag="d2tmp")
            nc.gpsimd.tensor_tensor(
                d2tmp[:].rearrange("p (m j) -> p m j", m=EDIM),
                e2[:].rearrange("p (m j) -> p m j", m=EDIM),
                p2B[:, :, None].to_broadcast([N, EDIM, N]),
                A.mult,
            )
            d2 = sb.tile([N, N], f32, tag="d2")
            nc.vector.tensor_reduce(
                d2[:], d2tmp[:].rearrange("p (m j) -> p j m", m=EDIM),
                mybir.AxisListType.X, A.add,
            )
            A2 = sb.tile([N, N], f32, tag="A2")
            nc.gpsimd.tensor_tensor(A2[:], d2[:], mask, A.mult)

            # ---- node conv 2 (no relu) + mean pool + head ----
            xW2_ps = ps.tile([N, HID], f32, tag="ps")
            nc.tensor.matmul(xW2_ps[:], x1T[:], W2, start=True, stop=True)
            xW2 = sb.tile([N, HID], f32, tag="xW2")
            nc.vector.tensor_copy(xW2[:], xW2_ps[:])
            x2T_ps = ps.tile([HID, N], f32, tag="ps")
            nc.tensor.matmul(x2T_ps[:], xW2[:], A2[:], start=True, stop=True)
            red = sb.tile([HID, 1], f32, tag="red")
            nc.vector.tensor_reduce(red[:], x2T_ps[:], mybir.AxisListType.X, A.add)
            pooledT = sb.tile([HID, 1], f32, tag="pooledT")
            nc.vector.tensor_scalar(
                pooledT[:], red[:], 1.0 / N, b2, A.mult, A.add
            )
            outT_ps = ps.tile([OUT, 1], f32, tag="ps")
            nc.tensor.matmul(outT_ps[:], Wl, pooledT[:], start=True, stop=True)
            out_sb = sb.tile([OUT, 1], f32, tag="out_sb")
            nc.vector.tensor_scalar_add(out_sb[:], outT_ps[:], bl)
            nc.sync.dma_start(out=out_d[:], in_=out_sb[:])

    _split_excess_waits(nc, mybir)
    return nc


def _prep_inputs(inputs):
    ei = np.asarray(inputs["edge_index"][0], dtype=np.int64)
    ej = np.asarray(inputs["edge_index"][1], dtype=np.int64)
    ea = np.asarray(inputs["edge_attr"], dtype=np.float32)

    ea_dense = np.zeros((N, EDIM, N), dtype=np.float32)
    ea_dense[ei, :, ej] = ea
    ea_dense[ej, :, ei] = ea

    slab = np.zeros((128, SLAB_W), dtype=np.float32)
    slab[0:N, 0:EDIM * N] = ea_dense.reshape(N, EDIM * N)
    slab[0, C_SVEC:C_SVEC + 40] = np.concatenate(
        [
            np.asarray(inputs["p1"], dtype=np.float32).reshape(-1),
            np.asarray(inputs["p2"], dtype=np.float32).reshape(-1),
            np.asarray(inputs["be"], dtype=np.float32).reshape(-1),
            np.asarray(inputs["We"], dtype=np.float32).reshape(-1),
        ]
    )
    slab[0:ENC, C_ENCT:C_ENCT + N] = np.asarray(
        inputs["encoding_raw"], dtype=np.float32
    ).T
    slab[0:ENC, C_WENC:C_WENC + HID] = np.asarray(inputs["W_enc"], dtype=np.float32)
    slab[0:HID, C_W1:C_W1 + HID] = np.asarray(inputs["W1"], dtype=np.float32)
    slab[0:HID, C_W2:C_W2 + HID] = np.asarray(inputs["W2"], dtype=np.float32)
    slab[0:HID, C_WL:C_WL + OUT] = np.asarray(inputs["Wl"], dtype=np.float32)
    slab[0:N, C_MASK:C_MASK + N] = 1.0 - np.eye(N, dtype=np.float32)
    slab[0:HID, C_BENC] = np.asarray(inputs["b_enc"], dtype=np.float32).reshape(-1)
    slab[0:HID, C_B1] = np.asarray(inputs["b1"], dtype=np.float32).reshape(-1)
    slab[0:HID, C_B2] = np.asarray(inputs["b2"], dtype=np.float32).reshape(-1)
    slab[0:HID, C_PET] = np.asarray(inputs["pe"], dtype=np.float32).reshape(-1)
    slab[0:OUT, C_BL] = np.asarray(inputs["bl"], dtype=np.float32).reshape(-1)

    return {
        "slabA": np.ascontiguousarray(slab[:, 0:SPLIT]),
        "slabB": np.ascontiguousarray(slab[:, SPLIT:SLAB_W]),
    }


def kernel(**inputs) -> np.ndarray:
    import sys

    if "/opt/trn_rl_repo" not in sys.path:
        sys.path.insert(0, "/opt/trn_rl_repo")
    from concourse.bass_utils import run_bass_kernel_spmd

    if "nc" not in _CACHE:
        _CACHE["nc"] = _build()
    nc = _CACHE["nc"]

    in_map = _prep_inputs(inputs)
    res = run_bass_kernel_spmd(
        nc, [in_map] * 8, core_ids=list(range(8)), trace=False
    )
    return np.asarray(res.results[0]["out"], dtype=np.float32).reshape(1, OUT)


# revision 12
# speedup vs baseline: 1.0560x; 1.0560x over previous
"""Trainium2 Bass kernel for nn_Net_3152505995417 (gnn_message_passing).

Closed-form reformulation: with T the incidence matrix of a simple graph,
  node conv:  (T diag(d) T^T) * adj_v  ==  A with A[i,j] = d[edge(i,j)], 0 diag
  edge conv:  M = (T^T diag(dv) T) * adj_e has M[e,f] = dv[shared node],
              col-max(f=(k,l)) = max(dv[k], dv[l], 0)   (complete graph),
              row e=(i,j) of (M/colmax) @ G = dv_i*(S_i - Gn_e) + dv_j*(S_j - Gn_e)
              with Gn = G / (colmax + eps), S = T @ Gn.
So the E x E matrix is never materialized: everything lives in a dense
[N, N] node-pair layout (slots (i,j) and (j,i) both carry edge {i,j};
diagonal slots are zero). Gathers/scatters become row/column broadcasts
(PE ones-matmuls) and free-dim row-sums of [116, 116] tiles.

All inputs arrive in one packed [128, 1016] slab (two DMAs); the full
model runs replicated on each of the 8 NeuronCores (total work is a few
hundred KB — replication beats collective latency); core 0's output is
returned.
"""

import numpy as np

N = 116
E = N * (N - 1) // 2
HID = 64
EDIM = 5
OUT = 4
ENC = HID + N // 2
EPS = 1e-10

# packed slab column offsets
C_EA = 0                 # [0:116, 0:580]   ea dense, k-major (f = k*N + j)
C_SVEC = 1016            # [0, 1016:1056]   p1|p2|be|We.flat
C_ENCT = 580             # [0:122, 580:696]
C_WENC = 696             # [0:122, 696:760]
C_W1 = 760               # [0:64]
C_W2 = 824
C_WL = 888               # [0:64, 888:892]
C_MASK = 892             # [0:116, 892:1008]
C_BENC = 1008
C_B1 = 1009
C_B2 = 1010
C_PET = 1011
C_BL = 1012              # [0:4]
SLAB_W = 1056
SPLIT = 580              # DMA A = cols [0:580], DMA B = cols [580:1016]

# plane-boundary chunking of the 5*116 edge-conv slab (PE N<=512, PSUM bank)
CH = [(0, 232), (232, 580)]

_CACHE = {}


def _split_excess_waits(nc, mybir, max_waits=1):
    """Workaround: this walrus build accepts only one sync-wait per
    instruction (setupSyncWait: "Too many sync wait commands"). Move excess
    waits onto chained NoOps on the same engine immediately before the
    instruction; sequencer semantics are unchanged."""
    for fn in nc.m.functions:
        for blk in fn.blocks:
            insts = blk.instructions
            new, changed = [], False
            for ins in insts:
                si = ins.sync_info
                waits = list(si.on_wait) if si is not None else []
                if len(waits) > max_waits:
                    while len(waits) > max_waits:
                        chunk, waits = waits[:1], waits[1:]
                        nop = mybir.InstNoOp(
                            name=nc.get_next_instruction_name(),
                            engine=ins.engine,
                            sync_info=mybir.SyncInfo(on_wait=chunk, on_update=[]),
                            bass_nofuse=True,
                        )
                        new.append(nop)
                    si.on_wait = waits
                    changed = True
                new.append(ins)
            if changed:
                blk.instructions = new


def _build():
    import concourse.bass as bass
    import concourse.tile as tile
    from concourse import mybir

    f32 = mybir.dt.float32
    A = mybir.AluOpType
    Relu = mybir.ActivationFunctionType.Relu

    nc = bass.Bass("TRN2", target_bir_lowering=False, num_devices=8)

    slabA_d = nc.declare_dram_parameter("slabA", [128, SPLIT], f32, isOutput=False)
    slabB_d = nc.declare_dram_parameter(
        "slabB", [128, SLAB_W - SPLIT], f32, isOutput=False
    )
    out_d = nc.declare_dram_parameter("out", [OUT, 1], f32, isOutput=True)

    with tile.TileContext(nc) as tc:
        with (
            tc.tile_pool(name="sb", bufs=1) as sb,
            tc.tile_pool(name="pm", bufs=2) as pm,
            tc.tile_pool(name="ps", bufs=3, space="PSUM") as ps,
            tc.tile_pool(name="ps2", bufs=2, space="PSUM") as ps2,
        ):
            slab = sb.tile([128, SLAB_W], f32, tag="slab")
            nc.sync.dma_start(out=slab[:, SPLIT:SLAB_W], in_=slabB_d[:])
            nc.sync.dma_start(out=slab[:, 0:SPLIT], in_=slabA_d[:])

            ea = slab[0:N, 0:EDIM * N]
            svec = slab[0:1, C_SVEC:C_SVEC + 40]
            encT = slab[0:ENC, C_ENCT:C_ENCT + N]
            Wenc = slab[0:ENC, C_WENC:C_WENC + HID]
            W1 = slab[0:HID, C_W1:C_W1 + HID]
            W2 = slab[0:HID, C_W2:C_W2 + HID]
            Wl = slab[0:HID, C_WL:C_WL + OUT]
            mask = slab[0:N, C_MASK:C_MASK + N]
            benc = slab[0:HID, C_BENC:C_BENC + 1]
            b1 = slab[0:HID, C_B1:C_B1 + 1]
            b2 = slab[0:HID, C_B2:C_B2 + 1]
            peT = slab[0:HID, C_PET:C_PET + 1]
            bl = slab[0:OUT, C_BL:C_BL + 1]

            ones_row = sb.tile([1, N], f32, tag="ones_row")
            nc.vector.memset(ones_row[:], 1.0)
            ones_col = sb.tile([N, 1], f32, tag="ones_col")
            nc.vector.memset(ones_col[:], 1.0)
            # pre-warm the ACT Relu table while DMAs are in flight
            warm = sb.tile([1, 1], f32, tag="warm")
            nc.scalar.activation(warm[:], ones_row[:, 0:1], Relu)
            zerosT = sb.tile([N, N], f32, tag="zerosT")
            nc.vector.memset(zerosT[:], 0.0)

            # ---- broadcast the small row-vector params to all partitions ----
            svecB_ps = ps.tile([N, 40], f32, tag="ps")
            nc.tensor.matmul(svecB_ps[:], ones_row[:], svec, start=True, stop=True)
            svecB = sb.tile([N, 40], f32, tag="svecB")
            nc.vector.tensor_copy(svecB[:], svecB_ps[:])
            p1B = svecB[:, 0:5]
            p2B = svecB[:, 5:10]
            beB = svecB[:, 10:15]
            # We[k, m] at column 15 + k*5 + m

            # ---- x = enc @ W_enc + b_enc  (kept transposed: [HID, N]) ----
            xT_ps = ps.tile([HID, N], f32, tag="ps")
            nc.tensor.matmul(xT_ps[:], Wenc, encT, start=True, stop=True)
            xT = sb.tile([HID, N], f32, tag="xT")
            nc.vector.tensor_scalar_add(xT[:], xT_ps[:], benc)

            # ---- A1 = d1 (dense pair layout; diag slots already zero) ----
            d1tmp = sb.tile([N, EDIM * N], f32, tag="d1tmp")
            nc.vector.tensor_tensor(
                d1tmp[:].rearrange("p (k j) -> p k j", k=EDIM),
                ea.rearrange("p (k j) -> p k j", k=EDIM),
                p1B[:, :, None].to_broadcast([N, EDIM, N]),
                A.mult,
            )
            d1 = sb.tile([N, N], f32, tag="d1")
            nc.vector.tensor_reduce(
                d1[:], d1tmp[:].rearrange("p (k j) -> p j k", k=EDIM),
                mybir.AxisListType.X, A.add,
            )

            # ---- node conv 1: x1T = relu((A1 @ (x @ W1) + b1)^T) ----
            xW1_ps = ps.tile([N, HID], f32, tag="ps")
            nc.tensor.matmul(xW1_ps[:], xT[:], W1, start=True, stop=True)
            xW1 = sb.tile([N, HID], f32, tag="xW1")
            nc.vector.tensor_copy(xW1[:], xW1_ps[:])
            x1T_ps = ps.tile([HID, N], f32, tag="ps")
            nc.tensor.matmul(x1T_ps[:], xW1[:], d1[:], start=True, stop=True)
            x1T = sb.tile([HID, N], f32, tag="x1T")
            nc.scalar.activation(x1T[:], x1T_ps[:], Relu, bias=b1)

            # ---- dv = x1 @ pe^T, as row [1,N] and column [N,1] ----
            dvr_ps = ps.tile([1, N], f32, tag="ps")
            nc.tensor.matmul(dvr_ps[:], peT, x1T[:], start=True, stop=True)
            dv_row = sb.tile([1, N], f32, tag="dv_row")
            nc.vector.tensor_copy(dv_row[:], dvr_ps[:])
            dv_rowE = sb.tile([1, N], f32, tag="dv_rowE")
            nc.vector.tensor_scalar_add(dv_rowE[:], dvr_ps[:], EPS)
            dvT_ps = ps.tile([N, 1], f32, tag="ps")
            nc.tensor.matmul(dvT_ps[:], x1T[:], peT, start=True, stop=True)
            dvTe = sb.tile([N, 1], f32, tag="dvTe")
            nc.vector.tensor_scalar_add(dvTe[:], dvT_ps[:], EPS)
            dvROW_ps = ps.tile([N, N], f32, tag="ps")
            # dvROWe[i,j] = dv_j + eps (the +eps rides along; max is shift-
            # invariant so cmeps = max(dv_i+eps, dv_j+eps, eps))
            nc.tensor.matmul(dvROW_ps[:], ones_row[:], dv_rowE[:], start=True, stop=True)

            # negsumdv[i,j] = -(dv_i + dv_j) (up to 2eps, far below f32 ulp)
            negsumdv = sb.tile([N, N], f32, tag="negsumdv")
            nc.vector.tensor_scalar(
                negsumdv[:], dvROW_ps[:], dvTe[:, 0:1], -1.0, A.add, A.mult
            )
            cmeps = sb.tile([N, N], f32, tag="cmeps")
            nc.vector.tensor_scalar(
                cmeps[:], dvROW_ps[:], dvTe[:, 0:1], EPS, A.max, A.max
            )
            nc.vector.reciprocal(cmeps[:], cmeps[:])

            # ---- edge conv (plane-major slabs, f = m*N + j) ----
            eR = sb.tile([N, EDIM * N], f32, tag="eR")
            nc.scalar.activation(eR[:], ea, Relu)

            # G[:, (m,j)] = sum_k eR_k[:, j] * We[k, m]; eR_k broadcast along
            # m (outer step-0), We row broadcast along j (inner step-0).
            def eRk_b(k):
                return eR[:, k * N:(k + 1) * N][:, None, :].to_broadcast(
                    [N, EDIM, N]
                )

            def WeB_b(k):
                return svecB[:, 15 + k * 5:15 + k * 5 + 5][:, :, None].to_broadcast(
                    [N, EDIM, N]
                )

            G = sb.tile([N, EDIM * N], f32, tag="G")
            G3 = G[:].rearrange("p (m j) -> p m j", m=EDIM)
            Gt = sb.tile([N, EDIM * N], f32, tag="Gt")
            Gt3 = Gt[:].rearrange("p (m j) -> p m j", m=EDIM)
            Gg = sb.tile([N, EDIM * N], f32, tag="Gg")
            Gg3 = Gg[:].rearrange("p (m j) -> p m j", m=EDIM)
            nc.vector.tensor_tensor(G3, eRk_b(0), WeB_b(0), A.mult)
            for k in (1, 2, 3):
                nc.vector.tensor_tensor(Gt3, eRk_b(k), WeB_b(k), A.mult)
                nc.vector.tensor_tensor(G3, G3, Gt3, A.add)
            nc.gpsimd.tensor_tensor(Gg3, eRk_b(4), WeB_b(4), A.mult)
            nc.vector.tensor_tensor(G3, G3, Gg3, A.add)

            # Gn = G * (1/cmeps) and S[i,m] = sum_j Gn_m[i,j], per chunk so
            # the S_row/U/z chain for chunk 0 overlaps chunk 1's compute
            Gn = sb.tile([N, EDIM * N], f32, tag="Gn")
            S_all = sb.tile([N, EDIM], f32, tag="S_all")
            for c0, c1 in CH:
                nm = (c1 - c0) // N
                m0 = c0 // N
                nc.vector.tensor_tensor(
                    Gn[:, c0:c1].rearrange("p (m j) -> p m j", m=nm),
                    G[:, c0:c1].rearrange("p (m j) -> p m j", m=nm),
                    cmeps[:, None, :].to_broadcast([N, nm, N]),
                    A.mult,
                )
                nc.vector.tensor_reduce(
                    S_all[:, m0:m0 + nm],
                    Gn[:, c0:c1].rearrange("p (m j) -> p m j", m=nm),
                    mybir.AxisListType.X, A.add,
                )
            # t1b[:, m] = dv_i * S[i, m] + be_m
            t1b = sb.tile([N, EDIM], f32, tag="t1b")
            nc.vector.scalar_tensor_tensor(
                t1b[:], S_all[:], dvTe[:, 0:1], beB, A.mult, A.add
            )

            # q = Gn * negsumdv (broadcast along m) -- on GpSimd, per chunk
            q = sb.tile([N, EDIM * N], f32, tag="q")
            for c0, c1 in CH:
                nm = (c1 - c0) // N
                nc.gpsimd.tensor_tensor(
                    q[:, c0:c1].rearrange("p (m j) -> p m j", m=nm),
                    Gn[:, c0:c1].rearrange("p (m j) -> p m j", m=nm),
                    negsumdv[:, None, :].to_broadcast([N, nm, N]),
                    A.mult,
                )

            # S as rows (colsum of symmetric Gn), then U[i, (m,j)] = dv_j*S[j,m]
            z = sb.tile([N, EDIM * N], f32, tag="z")
            for c0, c1 in CH:
                w = c1 - c0
                nm = w // N
                Srow_ps = ps2.tile([1, w], f32, tag="psrow")
                nc.tensor.matmul(
                    Srow_ps[:], ones_col[:], Gn[:, c0:c1], start=True, stop=True
                )
                u = pm.tile([1, w], f32, tag="u")
                nc.vector.tensor_tensor(
                    u[:].rearrange("p (m j) -> p m j", m=nm),
                    dv_row[:, None, :].to_broadcast([1, nm, N]),
                    Srow_ps[:].rearrange("p (m j) -> p m j", m=nm),
                    A.mult,
                )
                U_ps = ps2.tile([N, w], f32, tag="psU")
                nc.tensor.matmul(U_ps[:], ones_row[:], u[:], start=True, stop=True)
                nc.vector.tensor_tensor(z[:, c0:c1], q[:, c0:c1], U_ps[:], A.add)

            # e2_m = relu(z_m + t1b_m): split across ACT and DVE
            e2 = sb.tile([N, EDIM * N], f32, tag="e2")
            for m in range(EDIM):
                sl = slice(m * N, (m + 1) * N)
                if m in (0, 1, 2):
                    nc.scalar.activation(
                        e2[:, sl], z[:, sl], Relu, bias=t1b[:, m:m + 1]
                    )
                else:
                    nc.vector.scalar_tensor_tensor(
                        e2[:, sl], z[:, sl], t1b[:, m:m + 1], zerosT[:],
                        A.add, A.max,
                    )

            # ---- A2 = (e2 @ p2^T) * mask ----
            d2tmp = sb.tile([N, EDIM * N], f32, tag="d2tmp")
            nc.vector.tensor_tensor(
                d2tmp[:].rearrange("p (m j) -> p m j", m=EDIM),
                e2[:].rearrange("p (m j) -> p m j", m=EDIM),
                p2B[:, :, None].to_broadcast([N, EDIM, N]),
                A.mult,
            )
            d2 = sb.tile([N, N], f32, tag="d2")
            nc.vector.tensor_reduce(
                d2[:], d2tmp[:].rearrange("p (m j) -> p j m", m=EDIM),
                mybir.AxisListType.X, A.add,
            )
            A2 = sb.tile([N, N], f32, tag="A2")
            nc.vector.tensor_tensor(A2[:], d2[:], mask, A.mult)

            # ---- node conv 2 (no relu) + mean pool + head ----
            xW2_ps = ps.tile([N, HID], f32, tag="ps")
            nc.tensor.matmul(xW2_ps[:], x1T[:], W2, start=True, stop=True)
            xW2 = sb.tile([N, HID], f32, tag="xW2")
            nc.vector.tensor_copy(xW2[:], xW2_ps[:])
            x2T_ps = ps.tile([HID, N], f32, tag="ps")
            nc.tensor.matmul(x2T_ps[:], xW2[:], A2[:], start=True, stop=True)
            red = sb.tile([HID, 1], f32, tag="red")
            nc.vector.tensor_reduce(red[:], x2T_ps[:], mybir.AxisListType.X, A.add)
            pooledT = sb.tile([HID, 1], f32, tag="pooledT")
            nc.vector.tensor_scalar(
                pooledT[:], red[:], 1.0 / N, b2, A.mult, A.add
            )
            outT_ps = ps.tile([OUT, 1], f32, tag="ps")
            nc.tensor.matmul(outT_ps[:], Wl, pooledT[:], start=True, stop=True)
            out_sb = sb.tile([OUT, 1], f32, tag="out_sb")
            nc.vector.tensor_scalar_add(out_sb[:], outT_ps[:], bl)
            nc.sync.dma_start(out=out_d[:], in_=out_sb[:])

    _split_excess_waits(nc, mybir)
    return nc


def _prep_inputs(inputs):
    ei = np.asarray(inputs["edge_index"][0], dtype=np.int64)
    ej = np.asarray(inputs["edge_index"][1], dtype=np.int64)
    ea = np.asarray(inputs["edge_attr"], dtype=np.float32)

    ea_dense = np.zeros((N, EDIM, N), dtype=np.float32)
    ea_dense[ei, :, ej] = ea
    ea_dense[ej, :, ei] = ea

    slab = np.zeros((128, SLAB_W), dtype=np.float32)
    slab[0:N, 0:EDIM * N] = ea_dense.reshape(N, EDIM * N)
    slab[0, C_SVEC:C_SVEC + 40] = np.concatenate(
        [
            np.asarray(inputs["p1"], dtype=np.float32).reshape(-1),
            np.asarray(inputs["p2"], dtype=np.float32).reshape(-1),
            np.asarray(inputs["be"], dtype=np.float32).reshape(-1),
            np.asarray(inputs["We"], dtype=np.float32).reshape(-1),
        ]
    )
    slab[0:ENC, C_ENCT:C_ENCT + N] = np.asarray(
        inputs["encoding_raw"], dtype=np.float32
    ).T
    slab[0:ENC, C_WENC:C_WENC + HID] = np.asarray(inputs["W_enc"], dtype=np.float32)
    slab[0:HID, C_W1:C_W1 + HID] = np.asarray(inputs["W1"], dtype=np.float32)
    slab[0:HID, C_W2:C_W2 + HID] = np.asarray(inputs["W2"], dtype=np.float32)
    slab[0:HID, C_WL:C_WL + OUT] = np.asarray(inputs["Wl"], dtype=np.float32)
    slab[0:N, C_MASK:C_MASK + N] = 1.0 - np.eye(N, dtype=np.float32)
    slab[0:HID, C_BENC] = np.asarray(inputs["b_enc"], dtype=np.float32).reshape(-1)
    slab[0:HID, C_B1] = np.asarray(inputs["b1"], dtype=np.float32).reshape(-1)
    slab[0:HID, C_B2] = np.asarray(inputs["b2"], dtype=np.float32).reshape(-1)
    slab[0:HID, C_PET] = np.asarray(inputs["pe"], dtype=np.float32).reshape(-1)
    slab[0:OUT, C_BL] = np.asarray(inputs["bl"], dtype=np.float32).reshape(-1)

    return {
        "slabA": np.ascontiguousarray(slab[:, 0:SPLIT]),
        "slabB": np.ascontiguousarray(slab[:, SPLIT:SLAB_W]),
    }


def kernel(**inputs) -> np.ndarray:
    import sys

    if "/opt/trn_rl_repo" not in sys.path:
        sys.path.insert(0, "/opt/trn_rl_repo")
    from concourse.bass_utils import run_bass_kernel_spmd

    if "nc" not in _CACHE:
        _CACHE["nc"] = _build()
    nc = _CACHE["nc"]

    in_map = _prep_inputs(inputs)
    res = run_bass_kernel_spmd(
        nc, [in_map] * 8, core_ids=list(range(8)), trace=False
    )
    return np.asarray(res.results[0]["out"], dtype=np.float32).reshape(1, OUT)
